# revision 25
# baseline (speedup 1.0000x reference)
"""Trainium2 kernel for nn_Attention_26774826124067.

Math: the reference module's score einsum sums heads out ('bqhe,bkhe->bqk')
and its value einsum sums the key axis out of the probabilities
('bqk,bqhe->bqhe').  Softmax rows sum to 1, so z == V exactly and the whole
module collapses to

    out[b,q,:] = x[b,q,:] @ M + bo,   M = Wv2 @ Wo2  (D x D),
    bo = b_O + b_V_flat @ Wo2

independent of W_Q/W_K/b_Q/b_K.  M and bo are tiny weight-only transforms,
folded on the host (fp32), so the device kernel is a single GEMM
out = x @ M sharded by ROWS across the 8 NeuronCores: core i computes
    outT_i = (x[i*1024:(i+1)*1024, :] @ M + bo)^T     (2048 x 1024)
with no collectives.  Mixed precision: contraction chunks 0..13 run in
bf16, chunks 14..15 as a single fp8(e4m3) DoubleRow matmul (contraction
256 in one PE pass), all accumulating in fp32 PSUM.  The fp8 operands are
pre-scaled (x*16, M*512, both well inside e4m3's +-240 range) and the bf16
M carries the same combined 8192 factor so every partial matches; the
PSUM->SBUF copies divide it back out.  Measured rel_l2 1.35e-2 vs the
2e-2 gate (deterministic -- fixed inputs), ~6.4us faster than all-bf16.

Schedule per core (timeline facts from ntff profiles):
- 32 dummy N=128 warm-up matmuls on a scratch tile run during the ~4.5us
  DMA trigger->data window, lifting the PE HAM clock-gate (1.2->2.4 GHz)
  right as real data lands.
- x and M are host-swizzled into [half][k-chunk] order so every transfer
  is contiguous >=2KB per partition (1KB-descriptor transfers measured
  only ~250 GB/s vs ~360 GB/s for 4KB+); transfer sizes are graduated --
  small k-chunks first for startup latency, 1-2MB quads later.
- Two k-outer passes over col-tiles 0..7 (row half 0, then half 1) keep
  the startup DMA demand at ~220 GB/s so the PE never starves while x
  streams in; then a c-outer steady state (x SBUF-resident) for 8..15.
- PSUM banks rotate so `start=True` never waits on a copy.
- The final row-slice of the last col-tile is split into four N=128
  accumulation groups in four long-free PSUM banks, so the tail after the
  last matmul is one tiny copy + a 32KB DMA instead of a 128KB block.
"""

import numpy as np
import ml_dtypes

import concourse.bass as bass  # noqa: F401  (engine types come via bacc)
import concourse.bacc as bacc
import concourse.mybir as mybir
from concourse.tile import TileContext
from concourse.bass_utils import run_bass_kernel_spmd

B, S, D, H, DH = 2, 4096, 2048, 16, 128
N_CORES = 8
P = 128
ROWS = B * S              # 8192
RPC = ROWS // N_CORES     # 1024 rows per core
KCH = D // P              # 16 contraction chunks
CT = D // P               # 16 output col-tiles of 128
RB = 512                  # matmul free dim (PSUM bank limit for f32 out)
NR = RPC // RB            # 2 row slices per core
PH1_C = 8                 # col-tiles handled in the k-outer passes
QN = 4                    # N=128 sub-groups in the final row-slice
MG = D // 2               # m col-group width (1024 = col-tiles 0..7 / 8..15)
KB = 14                   # bf16 k-chunks; chunks 14,15 run as one fp8 DoubleRow
SX, SM = 16.0, 512.0      # fp8 quantization scales for x and M
SCALE = SX * SM           # all PSUM partials carry this factor; copies divide

# Graduated k-chunk grouping for the startup stream: small first (latency),
# big later (descriptor efficiency / throughput).
KGROUPS = [(0, 1), (1, 2), (2, 3), (3, 4), (4, 6), (6, 8), (8, 12), (12, 16)]

_BF16 = ml_dtypes.bfloat16


def _build_nc():
    f32 = mybir.dt.float32
    bf16 = mybir.dt.bfloat16
    ident = mybir.ActivationFunctionType.Identity
    nc = bacc.Bacc(None, target_bir_lowering=False, debug=False)

    # Host-swizzled layouts (see prepare_in_maps):
    #   xh[p, h*KCH*RB + k*RB + j] = x[core_rows][k*128+p, h*RB+j]
    #   mh[p, g*KCH*MG + k*MG + c] = M[k*128+p, g*MG+c]
    xh = nc.declare_dram_parameter("xh", [P, NR * KCH * RB], bf16, isOutput=False)
    mh = nc.declare_dram_parameter("mh", [P, 2 * KCH * MG], bf16, isOutput=False)
    m8 = nc.declare_dram_parameter("m8", [P, 2 * D], mybir.dt.float8e4, isOutput=False)
    x8 = nc.declare_dram_parameter("x8", [P, 2 * RPC], mybir.dt.float8e4, isOutput=False)
    bo = nc.declare_dram_parameter("bo", [P, CT], f32, isOutput=False)
    out = nc.declare_dram_parameter("out", [D, RPC], bf16, isOutput=True)

    xh_r = xh[:].rearrange("p (h k j) -> p h k j", h=NR, k=KCH)  # [128,2,16,512]
    mh_r = mh[:].rearrange("p (g k c) -> p g k c", g=2, k=KCH)   # [128,2,16,1024]
    m8_r = m8[:].rearrange("p (i c) -> p i c", i=2)              # [128,2,2048]
    x8_r = x8[:].rearrange("p (i r) -> p i r", i=2)              # [128,2,1024]

    with TileContext(nc) as tc:
        with (
            tc.tile_pool(name="const", bufs=1) as const_pool,
            tc.tile_pool(name="obA", bufs=1) as outA_pool,
            tc.tile_pool(name="obB", bufs=3) as outB_pool,
            tc.tile_pool(name="ps", bufs=1, space="PSUM") as ps_pool,
        ):
            warm = const_pool.tile([P, P], bf16)
            bo_sb = const_pool.tile([P, CT], f32)
            x_sb = const_pool.tile([P, NR, KCH, RB], bf16)
            m_sb = const_pool.tile([P, 2, KCH, MG], bf16)
            m8_sb = const_pool.tile([P, 2, D], mybir.dt.float8e4)
            x8_sb = const_pool.tile([P, 2, RPC], mybir.dt.float8e4)

            def m_tile(c, k):
                return m_sb[:, c // PH1_C, k, (c % PH1_C) * P:(c % PH1_C + 1) * P]

            def dr_mm(pst, c, h, j0, j1):
                # fp8 DoubleRow pass over k-chunks 14,15 (contraction 256 in
                # one array pass), closing the accumulation group.
                nc.tensor.matmul(
                    pst,
                    m8_sb[:, :, c * P:(c + 1) * P],
                    x8_sb[:, :, h * RB + j0:h * RB + j1],
                    start=False,
                    stop=True,
                    perf_mode=mybir.MatmulPerfMode.DoubleRow,
                )

            def copy_v(dst, src, c):
                nc.vector.tensor_scalar(
                    dst, src, 1.0 / SCALE, bo_sb[:, c:c + 1],
                    mybir.AluOpType.mult, mybir.AluOpType.add,
                )

            def copy_s(dst, src, c):
                nc.scalar.activation(
                    dst, src, ident, bias=bo_sb[:, c:c + 1], scale=1.0 / SCALE
                )

            nc.vector.memset(warm[:], 0.0)
            # Bulk input stream on the sync ring (two active rings HALVE
            # per-ring throughput -- measured), but the first two m k-chunks
            # ride the otherwise-idle scalar ring so both rings' cold-start
            # ramps overlap and the first matmul's operands land ~1.7us
            # earlier.  Strict first-need order, graduated chunk sizes.
            for h0, h1 in ((0, PH1_C // 2 * P), (PH1_C // 2 * P, MG)):
                nc.scalar.dma_start(
                    out=m_sb[:, 0, 0, h0:h1], in_=mh_r[:, 0, 0, h0:h1]
                )
            nc.sync.dma_start(out=x_sb[:, 0, 0:1, :], in_=xh_r[:, 0, 0:1, :])
            nc.sync.dma_start(out=m_sb[:, 0, 1:2, :], in_=mh_r[:, 0, 1:2, :])
            nc.sync.dma_start(out=x_sb[:, 0, 1:2, :], in_=xh_r[:, 0, 1:2, :])
            for k0, k1 in ((2, 3), (3, 4), (4, 6), (6, 8), (8, 11), (11, 14)):
                nc.sync.dma_start(out=m_sb[:, 0, k0:k1, :], in_=mh_r[:, 0, k0:k1, :])
                nc.sync.dma_start(out=x_sb[:, 0, k0:k1, :], in_=xh_r[:, 0, k0:k1, :])
            nc.sync.dma_start(out=m8_sb[:], in_=m8_r[:])
            nc.sync.dma_start(out=x8_sb[:], in_=x8_r[:])
            for k0, k1 in ((0, 8), (8, 14)):
                nc.sync.dma_start(out=x_sb[:, 1, k0:k1, :], in_=xh_r[:, 1, k0:k1, :])
            nc.sync.dma_start(out=bo_sb[:], in_=bo[:])
            for k0, k1 in ((0, 8), (8, 14)):
                nc.sync.dma_start(out=m_sb[:, 1, k0:k1, :], in_=mh_r[:, 1, k0:k1, :])

            # PSUM bank map (8 banks).  Pass 1a: (c,0)->bank c; pass 1b:
            # (c,1)->bank c (freed by 1a's copy ~1us before reuse).  Phase
            # 2 c=8..14: (c,r)->bank (2(c-8)+r)%8; c=15: r0->bank 6, r1 in
            # four N=128 quarters on banks 7,0,1,2 (all long free).
            ps = {}
            for c in range(PH1_C):
                ps[(c, 0)] = ps_pool.tile(
                    [P, RB], f32, name=f"psA{c}", tag=f"bank{c}", bufs=1
                )
                ps[(c, 1)] = ps_pool.tile(
                    [P, RB], f32, name=f"psB{c}", tag=f"bank{c}", bufs=1
                )
            for c in range(PH1_C, CT - 1):
                for r in range(NR):
                    ps[(c, r)] = ps_pool.tile(
                        [P, RB], f32, name=f"ps{c}_{r}",
                        tag=f"bank{(2 * (c - PH1_C) + r) % 8}", bufs=1,
                    )
            ps[(CT - 1, 0)] = ps_pool.tile(
                [P, RB], f32, name=f"ps{CT - 1}_0", tag="bank6", bufs=1
            )
            psq = [
                ps_pool.tile(
                    [P, P], f32, name=f"psq{q}", tag=f"bank{(7 + q) % 8}", bufs=1
                )
                for q in range(QN)
            ]
            warm_ps = ps_pool.tile([P, P], f32, name="warm", tag="bank5", bufs=1)
            for _ in range(32):
                nc.tensor.matmul(
                    warm_ps[:], warm[:], warm[:], start=True, stop=True
                )

            # Pass 1a/1b: k-outer over col-tiles 0..7, one row half each.
            obs_a = {}
            for r in range(NR):
                for k in range(KB):
                    for c in range(PH1_C):
                        nc.tensor.matmul(
                            ps[(c, r)][:],
                            m_tile(c, k),
                            x_sb[:, r, k, :],
                            start=(k == 0),
                            stop=False,
                        )
                for c in range(PH1_C):
                    dr_mm(ps[(c, r)][:], c, r, 0, RB)
                for c in range(PH1_C):
                    if r == 0:
                        obs_a[c] = outA_pool.tile(
                            [P, RPC], bf16, name=f"obA{c}", tag=f"obA{c}"
                        )
                    ob = obs_a[c]
                    dst = ob[:, r * RB:(r + 1) * RB]
                    if c % 2 == 0:
                        copy_v(dst, ps[(c, r)][:], c)
                    else:
                        copy_s(dst, ps[(c, r)][:], c)
                    if r == NR - 1:
                        nc.scalar.dma_start(
                            out=out[c * P:(c + 1) * P, :], in_=ob[:]
                        )

            # Phase 2 (c-outer, x resident), col-tiles 8..14.
            for c in range(PH1_C, CT - 1):
                for k in range(KB):
                    for r in range(NR):
                        nc.tensor.matmul(
                            ps[(c, r)][:],
                            m_tile(c, k),
                            x_sb[:, r, k, :],
                            start=(k == 0),
                            stop=False,
                        )
                for r in range(NR):
                    dr_mm(ps[(c, r)][:], c, r, 0, RB)
                ob = outB_pool.tile([P, RPC], bf16, name=f"obB{c}", tag="obB")
                copy_v(ob[:, 0:RB], ps[(c, 0)][:], c)
                copy_s(ob[:, RB:RPC], ps[(c, 1)][:], c)
                nc.scalar.dma_start(out=out[c * P:(c + 1) * P, :], in_=ob[:])

            # Last col-tile: r0 as one N=512 group (its copy/DMA overlap the
            # quarter matmuls), r1 as four N=128 groups so the tail after
            # the very last matmul is a [128,128] copy + 32KB DMA.
            c = CT - 1
            ob = outB_pool.tile([P, RPC], bf16, name=f"obB{c}", tag="obB")
            for k in range(KB):
                nc.tensor.matmul(
                    ps[(c, 0)][:],
                    m_tile(c, k),
                    x_sb[:, 0, k, :],
                    start=(k == 0),
                    stop=False,
                )
            dr_mm(ps[(c, 0)][:], c, 0, 0, RB)
            copy_s(ob[:, 0:RB], ps[(c, 0)][:], c)
            nc.scalar.dma_start(out=out[c * P:(c + 1) * P, 0:RB], in_=ob[:, 0:RB])
            for q in range(QN):
                lo = RB + q * P
                for k in range(KB):
                    nc.tensor.matmul(
                        psq[q][:],
                        m_tile(c, k),
                        x_sb[:, 1, k, q * P:(q + 1) * P],
                        start=(k == 0),
                        stop=False,
                    )
                dr_mm(psq[q][:], c, 1, q * P, (q + 1) * P)
                copy_v(ob[:, lo:lo + P], psq[q][:], c)
                nc.scalar.dma_start(
                    out=out[c * P:(c + 1) * P, lo:lo + P], in_=ob[:, lo:lo + P]
                )
    nc.compile()
    return nc


_NC = None


def _get_nc():
    global _NC
    if _NC is None:
        _NC = _build_nc()
    return _NC


def prepare_in_maps(normalized_resid_pre, W_V, b_V, W_O, b_O):
    x2 = np.ascontiguousarray(
        np.asarray(normalized_resid_pre, dtype=np.float32).reshape(ROWS, D).T
    ).astype(_BF16)                                        # [D, ROWS]
    wv2 = np.asarray(W_V, dtype=np.float32).transpose(1, 0, 2).reshape(D, D)
    wo2 = np.asarray(W_O, dtype=np.float32).reshape(D, D)  # [h*e, d']
    m32 = wv2 @ wo2                                        # fused weight, [D, D]
    # All device partials carry the fp8 scale factor SX*SM: the bf16 M is
    # pre-multiplied so bf16 and fp8-DoubleRow matmuls accumulate into the
    # same PSUM banks consistently; copies divide by SCALE.
    m_bf = (m32 * SCALE).astype(_BF16)
    e4np = mybir.dt.np(mybir.dt.float8e4)
    # m8[p, i, c] = e4m3(M[(14+i)*128+p, c] * SM)
    m8 = np.ascontiguousarray(
        np.clip(m32[KB * P:, :] * SM, -240, 240)
        .reshape(2, P, D).transpose(1, 0, 2).reshape(P, -1)
        .astype(e4np)
    )
    # mh[p, g, k, c] = M[k*128+p, g*1024+c]
    mh = np.ascontiguousarray(
        m_bf.reshape(KCH, P, 2, MG).transpose(1, 2, 0, 3).reshape(P, -1)
    )
    bo_full = (
        np.asarray(b_O, dtype=np.float32)
        + np.asarray(b_V, dtype=np.float32).reshape(D) @ wo2
    )                                                      # [D]
    bo_sw = np.ascontiguousarray(bo_full.reshape(CT, P).T)  # [P, CT]
    x2f = np.asarray(normalized_resid_pre, dtype=np.float32).reshape(ROWS, D).T
    in_maps = []
    for i in range(N_CORES):
        xc = x2[:, i * RPC:(i + 1) * RPC]                  # [D, RPC]
        # xh[p, h, k, j] = xc[k*128+p, h*512+j]
        xhc = np.ascontiguousarray(
            xc.reshape(KCH, P, NR, RB).transpose(1, 2, 0, 3).reshape(P, -1)
        )
        # x8[p, i, r] = e4m3(x[core_rows][(14+i)*128+p, r] * SX)
        x8c = np.ascontiguousarray(
            np.clip(x2f[KB * P:, i * RPC:(i + 1) * RPC] * SX, -240, 240)
            .reshape(2, P, RPC).transpose(1, 0, 2).reshape(P, -1)
            .astype(e4np)
        )
        in_maps.append({"xh": xhc, "mh": mh, "m8": m8, "x8": x8c, "bo": bo_sw})
    return in_maps


def assemble_output(results):
    outT = np.concatenate(
        [np.asarray(r["out"]) for r in results], axis=1
    )  # [D, ROWS] bf16, bias already applied on device
    return np.ascontiguousarray(outT.T.astype(np.float32)).reshape(B, S, D)


def kernel(
    normalized_resid_pre,
    W_Q=None,
    b_Q=None,
    W_K=None,
    b_K=None,
    W_V=None,
    b_V=None,
    W_O=None,
    b_O=None,
    **_unused,
):
    nc = _get_nc()
    in_maps = prepare_in_maps(normalized_resid_pre, W_V, b_V, W_O, b_O)
    last_err = None
    for _attempt in range(3):
        try:
            res = run_bass_kernel_spmd(nc, in_maps, core_ids=list(range(N_CORES)))
            return assemble_output(res.results)
        except Exception as e:  # transient runtime hiccups: retry
            last_err = e
    raise last_err


if __name__ == "__main__":
    rng = np.random.default_rng(0)
    x = rng.standard_normal((B, S, D), dtype=np.float32)
    wq = rng.standard_normal((H, D, DH), dtype=np.float32) * 0.02
    wv = rng.standard_normal((H, D, DH), dtype=np.float32) * 0.02
    wo_ = rng.standard_normal((H, DH, D), dtype=np.float32) * 0.02
    out = kernel(
        x,
        W_Q=wq,
        b_Q=np.zeros((H, DH), np.float32),
        W_K=wq,
        b_K=np.zeros((H, DH), np.float32),
        W_V=wv,
        b_V=np.zeros((H, DH), np.float32),
        W_O=wo_,
        b_O=np.zeros((D,), np.float32),
    )
    expect = x.reshape(ROWS, D) @ (
        wv.transpose(1, 0, 2).reshape(D, D) @ wo_.reshape(D, D)
    )
    expect = expect.reshape(B, S, D)
    err = np.abs(out - expect).max() / np.abs(expect).max()
    print("quick self-check rel abs err:", err)


# revision 27
# speedup vs baseline: 1.0046x; 1.0046x over previous
"""Trainium2 kernel for nn_Attention_26774826124067.

Math: the reference module's score einsum sums heads out ('bqhe,bkhe->bqk')
and its value einsum sums the key axis out of the probabilities
('bqk,bqhe->bqhe').  Softmax rows sum to 1, so z == V exactly and the whole
module collapses to

    out[b,q,:] = x[b,q,:] @ M + bo,   M = Wv2 @ Wo2  (D x D),
    bo = b_O + b_V_flat @ Wo2

independent of W_Q/W_K/b_Q/b_K.  M and bo are tiny weight-only transforms,
folded on the host (fp32), so the device kernel is a single GEMM
out = x @ M sharded by ROWS across the 8 NeuronCores: core i computes
    outT_i = (x[i*1024:(i+1)*1024, :] @ M + bo)^T     (2048 x 1024)
with no collectives.  Mixed precision: contraction chunks 0..13 run in
bf16, chunks 14..15 as a single fp8(e4m3) DoubleRow matmul (contraction
256 in one PE pass), all accumulating in fp32 PSUM.  The fp8 operands are
pre-scaled (x*16, M*512, both well inside e4m3's +-240 range) and the bf16
M carries the same combined 8192 factor so every partial matches; the
PSUM->SBUF copies divide it back out.  Measured rel_l2 1.35e-2 vs the
2e-2 gate (deterministic -- fixed inputs), ~6.4us faster than all-bf16.

Schedule per core (timeline facts from ntff profiles):
- 32 dummy N=128 warm-up matmuls on a scratch tile run during the ~4.5us
  DMA trigger->data window, lifting the PE HAM clock-gate (1.2->2.4 GHz)
  right as real data lands.
- x and M are host-swizzled into [half][k-chunk] order so every transfer
  is contiguous >=2KB per partition (1KB-descriptor transfers measured
  only ~250 GB/s vs ~360 GB/s for 4KB+); transfer sizes are graduated --
  small k-chunks first for startup latency, 1-2MB quads later.
- Two k-outer passes over col-tiles 0..7 (row half 0, then half 1) keep
  the startup DMA demand at ~220 GB/s so the PE never starves while x
  streams in; then a c-outer steady state (x SBUF-resident) for 8..15.
- PSUM banks rotate so `start=True` never waits on a copy.
- The final row-slice of the last col-tile is split into four N=128
  accumulation groups in four long-free PSUM banks, so the tail after the
  last matmul is one tiny copy + a 32KB DMA instead of a 128KB block.
"""

import numpy as np
import ml_dtypes

import concourse.bass as bass  # noqa: F401  (engine types come via bacc)
import concourse.bacc as bacc
import concourse.mybir as mybir
from concourse.tile import TileContext
from concourse.bass_utils import run_bass_kernel_spmd

B, S, D, H, DH = 2, 4096, 2048, 16, 128
N_CORES = 8
P = 128
ROWS = B * S              # 8192
RPC = ROWS // N_CORES     # 1024 rows per core
KCH = D // P              # 16 contraction chunks
CT = D // P               # 16 output col-tiles of 128
RB = 512                  # matmul free dim (PSUM bank limit for f32 out)
NR = RPC // RB            # 2 row slices per core
PH1_C = 8                 # col-tiles handled in the k-outer passes
QN = 4                    # N=128 sub-groups in the final row-slice
MG = D // 2               # m col-group width (1024 = col-tiles 0..7 / 8..15)
KB = 14                   # bf16 k-chunks; chunks 14,15 run as one fp8 DoubleRow
SX, SM = 16.0, 512.0      # fp8 quantization scales for x and M
SCALE = SX * SM           # all PSUM partials carry this factor; copies divide

# Graduated k-chunk grouping for the startup stream: small first (latency),
# big later (descriptor efficiency / throughput).
KGROUPS = [(0, 1), (1, 2), (2, 3), (3, 4), (4, 6), (6, 8), (8, 12), (12, 16)]

_BF16 = ml_dtypes.bfloat16


def _build_nc():
    f32 = mybir.dt.float32
    bf16 = mybir.dt.bfloat16
    ident = mybir.ActivationFunctionType.Identity
    nc = bacc.Bacc(None, target_bir_lowering=False, debug=False)

    # Host-swizzled layouts (see prepare_in_maps):
    #   xh[p, h*KCH*RB + k*RB + j] = x[core_rows][k*128+p, h*RB+j]
    #   mh[p, g*KCH*MG + k*MG + c] = M[k*128+p, g*MG+c]
    xh = nc.declare_dram_parameter("xh", [P, NR * KCH * RB], bf16, isOutput=False)
    mh = nc.declare_dram_parameter("mh", [P, 2 * KCH * MG], bf16, isOutput=False)
    m8 = nc.declare_dram_parameter("m8", [P, 2 * D], mybir.dt.float8e4, isOutput=False)
    x8 = nc.declare_dram_parameter("x8", [P, 2 * RPC], mybir.dt.float8e4, isOutput=False)
    bo = nc.declare_dram_parameter("bo", [P, CT], f32, isOutput=False)
    out = nc.declare_dram_parameter("out", [D, RPC], bf16, isOutput=True)

    xh_r = xh[:].rearrange("p (h k j) -> p h k j", h=NR, k=KCH)  # [128,2,16,512]
    mh_r = mh[:].rearrange("p (g k c) -> p g k c", g=2, k=KCH)   # [128,2,16,1024]
    m8_r = m8[:].rearrange("p (i c) -> p i c", i=2)              # [128,2,2048]
    x8_r = x8[:].rearrange("p (i r) -> p i r", i=2)              # [128,2,1024]

    with TileContext(nc) as tc:
        with (
            tc.tile_pool(name="const", bufs=1) as const_pool,
            tc.tile_pool(name="obA", bufs=1) as outA_pool,
            tc.tile_pool(name="obB", bufs=3) as outB_pool,
            tc.tile_pool(name="ps", bufs=1, space="PSUM") as ps_pool,
        ):
            warm = const_pool.tile([P, P], bf16)
            bo_sb = const_pool.tile([P, CT], f32)
            x_sb = const_pool.tile([P, NR, KCH, RB], bf16)
            m_sb = const_pool.tile([P, 2, KCH, MG], bf16)
            m8_sb = const_pool.tile([P, 2, D], mybir.dt.float8e4)
            x8_sb = const_pool.tile([P, 2, RPC], mybir.dt.float8e4)

            def m_tile(c, k):
                return m_sb[:, c // PH1_C, k, (c % PH1_C) * P:(c % PH1_C + 1) * P]

            def dr_mm(pst, c, h, j0, j1):
                # fp8 DoubleRow pass over k-chunks 14,15 (contraction 256 in
                # one array pass), closing the accumulation group.
                nc.tensor.matmul(
                    pst,
                    m8_sb[:, :, c * P:(c + 1) * P],
                    x8_sb[:, :, h * RB + j0:h * RB + j1],
                    start=False,
                    stop=True,
                    perf_mode=mybir.MatmulPerfMode.DoubleRow,
                )

            def copy_v(dst, src, c):
                nc.vector.tensor_scalar(
                    dst, src, 1.0 / SCALE, bo_sb[:, c:c + 1],
                    mybir.AluOpType.mult, mybir.AluOpType.add,
                )

            def copy_s(dst, src, c):
                nc.scalar.activation(
                    dst, src, ident, bias=bo_sb[:, c:c + 1], scale=1.0 / SCALE
                )

            nc.vector.memset(warm[:], 0.0)
            # Bulk input stream on the sync ring (two active rings HALVE
            # per-ring throughput -- measured), but the first two m k-chunks
            # ride the otherwise-idle scalar ring so both rings' cold-start
            # ramps overlap and the first matmul's operands land ~1.7us
            # earlier.  Strict first-need order, graduated chunk sizes.
            # First k-chunk split across both rings' cold-start ramps: the
            # bigger m k0 rides the (faster-ramping) sync ring in halves,
            # the smaller x k0 rides the scalar ring.
            nc.scalar.dma_start(out=x_sb[:, 0, 0:1, :], in_=xh_r[:, 0, 0:1, :])
            for h0, h1 in ((0, PH1_C // 2 * P), (PH1_C // 2 * P, MG)):
                nc.sync.dma_start(
                    out=m_sb[:, 0, 0, h0:h1], in_=mh_r[:, 0, 0, h0:h1]
                )
            nc.sync.dma_start(out=m_sb[:, 0, 1:2, :], in_=mh_r[:, 0, 1:2, :])
            nc.sync.dma_start(out=x_sb[:, 0, 1:2, :], in_=xh_r[:, 0, 1:2, :])
            for k0, k1 in ((2, 3), (3, 4), (4, 6), (6, 8), (8, 11), (11, 14)):
                nc.sync.dma_start(out=m_sb[:, 0, k0:k1, :], in_=mh_r[:, 0, k0:k1, :])
                nc.sync.dma_start(out=x_sb[:, 0, k0:k1, :], in_=xh_r[:, 0, k0:k1, :])
            nc.sync.dma_start(out=m8_sb[:], in_=m8_r[:])
            nc.sync.dma_start(out=x8_sb[:], in_=x8_r[:])
            for k0, k1 in ((0, 8), (8, 14)):
                nc.sync.dma_start(out=x_sb[:, 1, k0:k1, :], in_=xh_r[:, 1, k0:k1, :])
            nc.sync.dma_start(out=bo_sb[:], in_=bo[:])
            for k0, k1 in ((0, 8), (8, 14)):
                nc.sync.dma_start(out=m_sb[:, 1, k0:k1, :], in_=mh_r[:, 1, k0:k1, :])

            # PSUM bank map (8 banks).  Pass 1a: (c,0)->bank c; pass 1b:
            # (c,1)->bank c (freed by 1a's copy ~1us before reuse).  Phase
            # 2 c=8..14: (c,r)->bank (2(c-8)+r)%8; c=15: r0->bank 6, r1 in
            # four N=128 quarters on banks 7,0,1,2 (all long free).
            ps = {}
            for c in range(PH1_C):
                ps[(c, 0)] = ps_pool.tile(
                    [P, RB], f32, name=f"psA{c}", tag=f"bank{c}", bufs=1
                )
                ps[(c, 1)] = ps_pool.tile(
                    [P, RB], f32, name=f"psB{c}", tag=f"bank{c}", bufs=1
                )
            for c in range(PH1_C, CT - 1):
                for r in range(NR):
                    ps[(c, r)] = ps_pool.tile(
                        [P, RB], f32, name=f"ps{c}_{r}",
                        tag=f"bank{(2 * (c - PH1_C) + r) % 8}", bufs=1,
                    )
            ps[(CT - 1, 0)] = ps_pool.tile(
                [P, RB], f32, name=f"ps{CT - 1}_0", tag="bank6", bufs=1
            )
            psq = [
                ps_pool.tile(
                    [P, P], f32, name=f"psq{q}", tag=f"bank{(7 + q) % 8}", bufs=1
                )
                for q in range(QN)
            ]
            warm_ps = ps_pool.tile([P, P], f32, name="warm", tag="bank5", bufs=1)
            for _ in range(28):
                nc.tensor.matmul(
                    warm_ps[:], warm[:], warm[:], start=True, stop=True
                )

            # Pass 1a/1b: k-outer over col-tiles 0..7, one row half each.
            obs_a = {}
            for r in range(NR):
                for k in range(KB):
                    for c in range(PH1_C):
                        nc.tensor.matmul(
                            ps[(c, r)][:],
                            m_tile(c, k),
                            x_sb[:, r, k, :],
                            start=(k == 0),
                            stop=False,
                        )
                for c in range(PH1_C):
                    dr_mm(ps[(c, r)][:], c, r, 0, RB)
                for c in range(PH1_C):
                    if r == 0:
                        obs_a[c] = outA_pool.tile(
                            [P, RPC], bf16, name=f"obA{c}", tag=f"obA{c}"
                        )
                    ob = obs_a[c]
                    dst = ob[:, r * RB:(r + 1) * RB]
                    if c % 2 == 0:
                        copy_v(dst, ps[(c, r)][:], c)
                    else:
                        copy_s(dst, ps[(c, r)][:], c)
                    if r == NR - 1:
                        nc.scalar.dma_start(
                            out=out[c * P:(c + 1) * P, :], in_=ob[:]
                        )

            # Phase 2 (c-outer, x resident), col-tiles 8..14.
            for c in range(PH1_C, CT - 1):
                for k in range(KB):
                    for r in range(NR):
                        nc.tensor.matmul(
                            ps[(c, r)][:],
                            m_tile(c, k),
                            x_sb[:, r, k, :],
                            start=(k == 0),
                            stop=False,
                        )
                for r in range(NR):
                    dr_mm(ps[(c, r)][:], c, r, 0, RB)
                ob = outB_pool.tile([P, RPC], bf16, name=f"obB{c}", tag="obB")
                copy_v(ob[:, 0:RB], ps[(c, 0)][:], c)
                copy_s(ob[:, RB:RPC], ps[(c, 1)][:], c)
                nc.scalar.dma_start(out=out[c * P:(c + 1) * P, :], in_=ob[:])

            # Last col-tile: r0 as one N=512 group (its copy/DMA overlap the
            # quarter matmuls), r1 as four N=128 groups so the tail after
            # the very last matmul is a [128,128] copy + 32KB DMA.
            c = CT - 1
            ob = outB_pool.tile([P, RPC], bf16, name=f"obB{c}", tag="obB")
            for k in range(KB):
                nc.tensor.matmul(
                    ps[(c, 0)][:],
                    m_tile(c, k),
                    x_sb[:, 0, k, :],
                    start=(k == 0),
                    stop=False,
                )
            dr_mm(ps[(c, 0)][:], c, 0, 0, RB)
            copy_s(ob[:, 0:RB], ps[(c, 0)][:], c)
            nc.scalar.dma_start(out=out[c * P:(c + 1) * P, 0:RB], in_=ob[:, 0:RB])
            for q in range(QN):
                lo = RB + q * P
                for k in range(KB):
                    nc.tensor.matmul(
                        psq[q][:],
                        m_tile(c, k),
                        x_sb[:, 1, k, q * P:(q + 1) * P],
                        start=(k == 0),
                        stop=False,
                    )
                dr_mm(psq[q][:], c, 1, q * P, (q + 1) * P)
                copy_v(ob[:, lo:lo + P], psq[q][:], c)
                nc.scalar.dma_start(
                    out=out[c * P:(c + 1) * P, lo:lo + P], in_=ob[:, lo:lo + P]
                )
    nc.compile()
    return nc


_NC = None


def _get_nc():
    global _NC
    if _NC is None:
        _NC = _build_nc()
    return _NC


def prepare_in_maps(normalized_resid_pre, W_V, b_V, W_O, b_O):
    x2 = np.ascontiguousarray(
        np.asarray(normalized_resid_pre, dtype=np.float32).reshape(ROWS, D).T
    ).astype(_BF16)                                        # [D, ROWS]
    wv2 = np.asarray(W_V, dtype=np.float32).transpose(1, 0, 2).reshape(D, D)
    wo2 = np.asarray(W_O, dtype=np.float32).reshape(D, D)  # [h*e, d']
    m32 = wv2 @ wo2                                        # fused weight, [D, D]
    # All device partials carry the fp8 scale factor SX*SM: the bf16 M is
    # pre-multiplied so bf16 and fp8-DoubleRow matmuls accumulate into the
    # same PSUM banks consistently; copies divide by SCALE.
    m_bf = (m32 * SCALE).astype(_BF16)
    e4np = mybir.dt.np(mybir.dt.float8e4)
    # m8[p, i, c] = e4m3(M[(14+i)*128+p, c] * SM)
    m8 = np.ascontiguousarray(
        np.clip(m32[KB * P:, :] * SM, -240, 240)
        .reshape(2, P, D).transpose(1, 0, 2).reshape(P, -1)
        .astype(e4np)
    )
    # mh[p, g, k, c] = M[k*128+p, g*1024+c]
    mh = np.ascontiguousarray(
        m_bf.reshape(KCH, P, 2, MG).transpose(1, 2, 0, 3).reshape(P, -1)
    )
    bo_full = (
        np.asarray(b_O, dtype=np.float32)
        + np.asarray(b_V, dtype=np.float32).reshape(D) @ wo2
    )                                                      # [D]
    bo_sw = np.ascontiguousarray(bo_full.reshape(CT, P).T)  # [P, CT]
    x2f = np.asarray(normalized_resid_pre, dtype=np.float32).reshape(ROWS, D).T
    in_maps = []
    for i in range(N_CORES):
        xc = x2[:, i * RPC:(i + 1) * RPC]                  # [D, RPC]
        # xh[p, h, k, j] = xc[k*128+p, h*512+j]
        xhc = np.ascontiguousarray(
            xc.reshape(KCH, P, NR, RB).transpose(1, 2, 0, 3).reshape(P, -1)
        )
        # x8[p, i, r] = e4m3(x[core_rows][(14+i)*128+p, r] * SX)
        x8c = np.ascontiguousarray(
            np.clip(x2f[KB * P:, i * RPC:(i + 1) * RPC] * SX, -240, 240)
            .reshape(2, P, RPC).transpose(1, 0, 2).reshape(P, -1)
            .astype(e4np)
        )
        in_maps.append({"xh": xhc, "mh": mh, "m8": m8, "x8": x8c, "bo": bo_sw})
    return in_maps


def assemble_output(results):
    outT = np.concatenate(
        [np.asarray(r["out"]) for r in results], axis=1
    )  # [D, ROWS] bf16, bias already applied on device
    return np.ascontiguousarray(outT.T.astype(np.float32)).reshape(B, S, D)


def kernel(
    normalized_resid_pre,
    W_Q=None,
    b_Q=None,
    W_K=None,
    b_K=None,
    W_V=None,
    b_V=None,
    W_O=None,
    b_O=None,
    **_unused,
):
    nc = _get_nc()
    in_maps = prepare_in_maps(normalized_resid_pre, W_V, b_V, W_O, b_O)
    last_err = None
    for _attempt in range(3):
        try:
            res = run_bass_kernel_spmd(nc, in_maps, core_ids=list(range(N_CORES)))
            return assemble_output(res.results)
        except Exception as e:  # transient runtime hiccups: retry
            last_err = e
    raise last_err


if __name__ == "__main__":
    rng = np.random.default_rng(0)
    x = rng.standard_normal((B, S, D), dtype=np.float32)
    wq = rng.standard_normal((H, D, DH), dtype=np.float32) * 0.02
    wv = rng.standard_normal((H, D, DH), dtype=np.float32) * 0.02
    wo_ = rng.standard_normal((H, DH, D), dtype=np.float32) * 0.02
    out = kernel(
        x,
        W_Q=wq,
        b_Q=np.zeros((H, DH), np.float32),
        W_K=wq,
        b_K=np.zeros((H, DH), np.float32),
        W_V=wv,
        b_V=np.zeros((H, DH), np.float32),
        W_O=wo_,
        b_O=np.zeros((D,), np.float32),
    )
    expect = x.reshape(ROWS, D) @ (
        wv.transpose(1, 0, 2).reshape(D, D) @ wo_.reshape(D, D)
    )
    expect = expect.reshape(B, S, D)
    err = np.abs(out - expect).max() / np.abs(expect).max()
    print("quick self-check rel abs err:", err)
